# revision 12
# baseline (speedup 1.0000x reference)
"""Trainium2 Bass kernel for EnhanceLayerLinear.

Computes out = GroupedLinear(Linear(x)):
    y = x @ W.T + b                      [B,S,D]
    out[..., g, :] = y[..., g, :] @ Wg[g].T + bg[g]   (block-diagonal, G groups)

Sharding: data-parallel over tokens (B*S = 8192 -> 1024 per core). Each core
runs both GEMM stages locally; the grouped stage shards trivially since it is
applied per token.

Stage 1 runs in bf16 (fp32 accumulate in psum): fp32r matmuls are limited to
~272 ns/MM by the 2-pass fp32 LDWEIGHTS (224 ns) that cannot hide behind a
213 ns matmul, and the hardware forbids mixing bf16 weights with fp32r
activations. Stage 2 (the small grouped matmul) runs in float32r - fp32
truncated to FP22 - directly off the psum evacuation, so y is never quantized
to bf16 and the grouped stage adds no extra rounding.

Layout trick: stage 1 computes y TRANSPOSED (features on partitions, tokens on
the free axis). That makes each 128-row psum tile exactly one group's slice
with the contraction axis of stage 2 already on partitions, so the grouped
matmul chains directly with zero on-chip transposes. The host hands the kernel
pre-transposed views of x / W / Wg and re-transposes the output.
"""

from collections import deque

import ml_dtypes
import numpy as np

import concourse.bacc as bacc
import concourse.bass as bass
import concourse.tile as tile
from concourse import mybir
from concourse import bass_utils

f32 = mybir.dt.float32
f32r = mybir.dt.float32r
bf16 = mybir.dt.bfloat16
ACT_ID = mybir.ActivationFunctionType.Identity

B, S, D = 4, 2048, 4096
T = B * S                 # 8192 tokens
G, IG = 32, 128           # groups x group size (4096 = 32*128)
NCORES = 8
TPC = T // NCORES         # 1024 tokens per core
KT = D // 128             # 32 contraction tiles
NMOV = 512                # moving free dim per matmul (= one psum bank of fp32)
NCH = TPC // NMOV         # 2 token chunks per core

_CACHE = {}


def _build():
    nc = bacc.Bacc("TRN2", target_bir_lowering=False, debug=False)
    # x_d[kt, tch, p, t] = x[core_t0 + tch*512 + t, kt*128 + p]   (xT half-tiles)
    # w_d[og, p, kt*128 + o] = W[og*128 + o, kt*128 + p]          (WT per out-group)
    # wg_d[i, g*128 + o] = Wg[g, o, i]                            (WgT)
    # b_d[i, g] = b[g*128 + i];  bg_d[o, g] = bg[g, o]
    x_d = nc.dram_tensor("x", [KT, NCH, 128, NMOV], bf16, kind="ExternalInput")
    w_d = nc.dram_tensor("w", [G, 128, D], bf16, kind="ExternalInput")
    wg_d = nc.dram_tensor("wg", [128, G * IG], f32r, kind="ExternalInput")
    b_d = nc.dram_tensor("b", [128, G], f32, kind="ExternalInput")
    bg_d = nc.dram_tensor("bg", [128, G], f32, kind="ExternalInput")
    # o_d[og, o, t] = out[core_t0 + t, og*128 + o]                (outT)
    o_d = nc.dram_tensor("o", [G, 128, TPC], f32, kind="ExternalOutput")

    with tile.TileContext(nc) as tc:
        with (
            tc.tile_pool(name="xp", bufs=KT * NCH) as xp,
            tc.tile_pool(name="wp", bufs=4) as wp,
            tc.tile_pool(name="cp", bufs=1) as cp,
            tc.tile_pool(name="yp", bufs=8) as yp,
            tc.tile_pool(name="op", bufs=3) as op,
            tc.tile_pool(name="ps1", bufs=3, space=bass.MemorySpace.PSUM) as ps1,
            tc.tile_pool(name="ps2", bufs=2, space=bass.MemorySpace.PSUM) as ps2,
        ):
            w_tiles = {}

            def load_w(key):
                t = wp.tile([128, D], bf16, tag="w")
                nc.sync.dma_start(t[:], w_d[key[1]])
                w_tiles[key] = t

            # DMA emission order matters for the ramp: first W[0] (gates the
            # first matmul), then the x half-tiles in consumption order, with
            # the small bias/Wg tensors (needed only ~30us in) interleaved
            # after the first chunk wave.
            b_sb = cp.tile([128, G], f32)
            nc.sync.dma_start(b_sb[:], b_d[:])
            load_w((0, 0))
            x_sb = [[None] * NCH for _ in range(KT)]
            # First x wave with the next W tiles and constants interleaved at
            # consumption-proportional points - the first ~35us of the kernel
            # is DMA-bandwidth-bound, so queue order here IS the schedule.
            wg_sb = cp.tile([128, G * IG], f32r)
            bg_sb = cp.tile([128, G], f32)
            for kt in range(KT):
                t = xp.tile([128, NMOV], bf16, tag="x")
                nc.sync.dma_start(t[:], x_d[kt, 0])
                x_sb[kt][0] = t
                if kt == 11:
                    load_w((0, 1))
                elif kt == 19:
                    load_w((0, 2))
                elif kt == 27:
                    load_w((0, 3))
            nc.sync.dma_start(wg_sb[:], wg_d[:])
            nc.sync.dma_start(bg_sb[:], bg_d[:])

            pending_q = deque()
            FLUSH_LAG = 4

            def flush_stage2(p):
                y_sb, og2, tch2 = p
                acc2 = ps2.tile([128, NMOV], f32, tag="acc2")
                nc.tensor.matmul(
                    acc2[:],
                    wg_sb[:, og2 * IG:(og2 + 1) * IG],
                    y_sb[:],
                    start=True,
                    stop=True,
                )
                o_sb = op.tile([128, NMOV], f32, tag="o")
                nc.scalar.activation(
                    o_sb[:], acc2[:], ACT_ID, bias=bg_sb[:, og2:og2 + 1]
                )
                nc.sync.dma_start(
                    o_d[og2][:, tch2 * NMOV:(tch2 + 1) * NMOV], o_sb[:]
                )

            # tch outer: the whole first token-chunk pass (32 groups,
            # ~220us of matmul) runs before any tch=1 tile is needed, so the
            # second x wave has enormous DMA slack. W streams twice; at bf16
            # that is still far below the per-core HBM budget.
            passes = [(tch, og) for tch in range(NCH) for og in range(G)]
            for idx, (tch, og) in enumerate(passes):
                w_sb = w_tiles.pop((tch, og))
                if idx + 4 < len(passes):
                    load_w(passes[idx + 4])
                # Trickle the second x wave in behind the W prefetches: two
                # 256 KB half-tiles per group keeps the W stream (needed in
                # ~4 groups) ahead of the x tiles (needed in ~30 groups).
                if idx < KT // 2:
                    for kt in (2 * idx, 2 * idx + 1):
                        t = xp.tile([128, NMOV], bf16, tag="x")
                        nc.sync.dma_start(t[:], x_d[kt, 1])
                        x_sb[kt][1] = t
                acc = ps1.tile([128, NMOV], f32, tag="acc")
                for kt in range(KT):
                    nc.tensor.matmul(
                        acc[:],
                        w_sb[:, kt * 128:(kt + 1) * 128],
                        x_sb[kt][tch][:],
                        start=(kt == 0),
                        stop=(kt == KT - 1),
                    )
                # Emit earlier iterations' grouped-stage matmuls here with a
                # lag: the ACT producers ran during previous groups (PE never
                # waits on the scalar engine) and the lag defers the first
                # use of wg past the DMA-bound ramp window.
                if len(pending_q) >= FLUSH_LAG:
                    flush_stage2(pending_q.popleft())
                y_sb = yp.tile([128, NMOV], f32r, tag="y")
                nc.scalar.activation(
                    y_sb[:], acc[:], ACT_ID, bias=b_sb[:, og:og + 1]
                )
                pending_q.append((y_sb, og, tch))
            while pending_q:
                flush_stage2(pending_q.popleft())

    nc.compile()
    return nc


def _get_nc():
    if "nc" not in _CACHE:
        _CACHE["nc"] = _build()
    return _CACHE["nc"]


def _run(x, W, b, Wg, bg, trace=False, tmpdir=None):
    x = np.ascontiguousarray(x, dtype=np.float32)
    W = np.ascontiguousarray(W, dtype=np.float32)
    b = np.ascontiguousarray(b, dtype=np.float32)
    Wg = np.ascontiguousarray(Wg, dtype=np.float32)
    bg = np.ascontiguousarray(bg, dtype=np.float32)

    # Host-side layout prep (pure permutes + weight casts, no math).
    # x: [B,S,D] -> per-core xT half-tiles [KT, NCH, 128, NMOV]
    x_dev = np.ascontiguousarray(
        x.reshape(NCORES, NCH, NMOV, KT, 128).transpose(0, 3, 1, 4, 2)
        .astype(ml_dtypes.bfloat16)
    )
    # W: [D_out, D_in] -> [og, p(k_local), kt*128 + o], bf16
    w_dev = np.ascontiguousarray(
        W.reshape(G, 128, KT, 128).transpose(0, 3, 2, 1).reshape(G, 128, D)
        .astype(ml_dtypes.bfloat16)
    )
    wg_dev = np.ascontiguousarray(
        Wg.transpose(2, 0, 1).reshape(128, G * IG)
    )
    b_dev = np.ascontiguousarray(b.reshape(G, 128).T)
    bg_dev = np.ascontiguousarray(bg.T)

    in_maps = [
        {"x": x_dev[c], "w": w_dev, "wg": wg_dev, "b": b_dev, "bg": bg_dev}
        for c in range(NCORES)
    ]
    nc = _get_nc()
    res = bass_utils.run_bass_kernel_spmd(
        nc, in_maps, core_ids=list(range(NCORES)), trace=trace, tmpdir=tmpdir
    )
    _CACHE["last_result"] = res

    out_t = np.concatenate(
        [res.results[c]["o"].reshape(D, TPC) for c in range(NCORES)], axis=1
    )
    return np.ascontiguousarray(out_t.T).reshape(B, S, D)


def kernel(x, W, b, Wg, bg):
    return _run(x, W, b, Wg, bg, trace=False)


# revision 13
# speedup vs baseline: 1.0051x; 1.0051x over previous
"""Trainium2 Bass kernel for EnhanceLayerLinear.

Computes out = GroupedLinear(Linear(x)):
    y = x @ W.T + b                      [B,S,D]
    out[..., g, :] = y[..., g, :] @ Wg[g].T + bg[g]   (block-diagonal, G groups)

Sharding: data-parallel over tokens (B*S = 8192 -> 1024 per core). Each core
runs both GEMM stages locally; the grouped stage shards trivially since it is
applied per token.

Both stages run in bf16 (fp32 accumulate in psum): fp32r matmuls are limited
to ~272 ns/MM by the 2-pass fp32 LDWEIGHTS (224 ns) that cannot hide behind a
213 ns matmul (the hardware also forbids mixing bf16 weights with fp32r
activations), and an f32r grouped-stage matmul costs a ~422 ns PE slot between
stage-1 groups where a bf16 one costs 216 ns.

Layout trick: stage 1 computes y TRANSPOSED (features on partitions, tokens on
the free axis). That makes each 128-row psum tile exactly one group's slice
with the contraction axis of stage 2 already on partitions, so the grouped
matmul chains directly with zero on-chip transposes. The host hands the kernel
pre-transposed views of x / W / Wg and re-transposes the output.
"""

from collections import deque

import ml_dtypes
import numpy as np

import concourse.bacc as bacc
import concourse.bass as bass
import concourse.tile as tile
from concourse import mybir
from concourse import bass_utils

f32 = mybir.dt.float32
f32r = mybir.dt.float32r
bf16 = mybir.dt.bfloat16
ACT_ID = mybir.ActivationFunctionType.Identity

B, S, D = 4, 2048, 4096
T = B * S                 # 8192 tokens
G, IG = 32, 128           # groups x group size (4096 = 32*128)
NCORES = 8
TPC = T // NCORES         # 1024 tokens per core
KT = D // 128             # 32 contraction tiles
NMOV = 512                # moving free dim per matmul (= one psum bank of fp32)
NCH = TPC // NMOV         # 2 token chunks per core

_CACHE = {}


def _build():
    nc = bacc.Bacc("TRN2", target_bir_lowering=False, debug=False)
    # x_d[kt, tch, p, t] = x[core_t0 + tch*512 + t, kt*128 + p]   (xT half-tiles)
    # w_d[og, p, kt*128 + o] = W[og*128 + o, kt*128 + p]          (WT per out-group)
    # wg_d[i, g*128 + o] = Wg[g, o, i]                            (WgT)
    # b_d[i, g] = b[g*128 + i];  bg_d[o, g] = bg[g, o]
    x_d = nc.dram_tensor("x", [KT, NCH, 128, NMOV], bf16, kind="ExternalInput")
    w_d = nc.dram_tensor("w", [G, 128, D], bf16, kind="ExternalInput")
    wg_d = nc.dram_tensor("wg", [128, G * IG], bf16, kind="ExternalInput")
    b_d = nc.dram_tensor("b", [128, G], f32, kind="ExternalInput")
    bg_d = nc.dram_tensor("bg", [128, G], f32, kind="ExternalInput")
    # o_d[og, o, t] = out[core_t0 + t, og*128 + o]                (outT)
    o_d = nc.dram_tensor("o", [G, 128, TPC], f32, kind="ExternalOutput")

    with tile.TileContext(nc) as tc:
        with (
            tc.tile_pool(name="xp", bufs=KT * NCH) as xp,
            tc.tile_pool(name="wp", bufs=4) as wp,
            tc.tile_pool(name="cp", bufs=1) as cp,
            tc.tile_pool(name="yp", bufs=8) as yp,
            tc.tile_pool(name="op", bufs=3) as op,
            tc.tile_pool(name="ps1", bufs=3, space=bass.MemorySpace.PSUM) as ps1,
            tc.tile_pool(name="ps2", bufs=2, space=bass.MemorySpace.PSUM) as ps2,
        ):
            w_tiles = {}

            def load_w(key):
                t = wp.tile([128, D], bf16, tag="w")
                nc.sync.dma_start(t[:], w_d[key[1]])
                w_tiles[key] = t

            # DMA emission order matters for the ramp: first W[0] (gates the
            # first matmul), then the x half-tiles in consumption order, with
            # the small bias/Wg tensors (needed only ~30us in) interleaved
            # after the first chunk wave.
            b_sb = cp.tile([128, G], f32)
            nc.sync.dma_start(b_sb[:], b_d[:])
            load_w((0, 0))
            x_sb = [[None] * NCH for _ in range(KT)]
            # First x wave with the next W tiles and constants interleaved at
            # consumption-proportional points - the first ~35us of the kernel
            # is DMA-bandwidth-bound, so queue order here IS the schedule.
            wg_sb = cp.tile([128, G * IG], bf16)
            bg_sb = cp.tile([128, G], f32)
            for kt in range(KT):
                t = xp.tile([128, NMOV], bf16, tag="x")
                nc.sync.dma_start(t[:], x_d[kt, 0])
                x_sb[kt][0] = t
                if kt == 11:
                    load_w((0, 1))
                elif kt == 19:
                    load_w((0, 2))
                elif kt == 27:
                    load_w((0, 3))
            nc.sync.dma_start(wg_sb[:], wg_d[:])
            nc.sync.dma_start(bg_sb[:], bg_d[:])

            pending_q = deque()
            FLUSH_LAG = 4

            def flush_stage2(p):
                y_sb, og2, tch2 = p
                acc2 = ps2.tile([128, NMOV], f32, tag="acc2")
                nc.tensor.matmul(
                    acc2[:],
                    wg_sb[:, og2 * IG:(og2 + 1) * IG],
                    y_sb[:],
                    start=True,
                    stop=True,
                )
                o_sb = op.tile([128, NMOV], f32, tag="o")
                nc.scalar.activation(
                    o_sb[:], acc2[:], ACT_ID, bias=bg_sb[:, og2:og2 + 1]
                )
                nc.sync.dma_start(
                    o_d[og2][:, tch2 * NMOV:(tch2 + 1) * NMOV], o_sb[:]
                )

            # tch outer: the whole first token-chunk pass (32 groups,
            # ~220us of matmul) runs before any tch=1 tile is needed, so the
            # second x wave has enormous DMA slack. W streams twice; at bf16
            # that is still far below the per-core HBM budget.
            passes = [(tch, og) for tch in range(NCH) for og in range(G)]
            for idx, (tch, og) in enumerate(passes):
                w_sb = w_tiles.pop((tch, og))
                if idx + 4 < len(passes):
                    load_w(passes[idx + 4])
                # Trickle the second x wave in behind the W prefetches: two
                # 256 KB half-tiles per group keeps the W stream (needed in
                # ~4 groups) ahead of the x tiles (needed in ~30 groups).
                if idx < KT // 2:
                    for kt in (2 * idx, 2 * idx + 1):
                        t = xp.tile([128, NMOV], bf16, tag="x")
                        nc.sync.dma_start(t[:], x_d[kt, 1])
                        x_sb[kt][1] = t
                acc = ps1.tile([128, NMOV], f32, tag="acc")
                for kt in range(KT):
                    nc.tensor.matmul(
                        acc[:],
                        w_sb[:, kt * 128:(kt + 1) * 128],
                        x_sb[kt][tch][:],
                        start=(kt == 0),
                        stop=(kt == KT - 1),
                    )
                # Emit earlier iterations' grouped-stage matmuls here with a
                # lag: the ACT producers ran during previous groups (PE never
                # waits on the scalar engine) and the lag defers the first
                # use of wg past the DMA-bound ramp window.
                if len(pending_q) >= FLUSH_LAG:
                    flush_stage2(pending_q.popleft())
                y_sb = yp.tile([128, NMOV], bf16, tag="y")
                nc.scalar.activation(
                    y_sb[:], acc[:], ACT_ID, bias=b_sb[:, og:og + 1]
                )
                pending_q.append((y_sb, og, tch))
            while pending_q:
                flush_stage2(pending_q.popleft())

    nc.compile()
    return nc


def _get_nc():
    if "nc" not in _CACHE:
        _CACHE["nc"] = _build()
    return _CACHE["nc"]


def _run(x, W, b, Wg, bg, trace=False, tmpdir=None):
    x = np.ascontiguousarray(x, dtype=np.float32)
    W = np.ascontiguousarray(W, dtype=np.float32)
    b = np.ascontiguousarray(b, dtype=np.float32)
    Wg = np.ascontiguousarray(Wg, dtype=np.float32)
    bg = np.ascontiguousarray(bg, dtype=np.float32)

    # Host-side layout prep (pure permutes + weight casts, no math).
    # x: [B,S,D] -> per-core xT half-tiles [KT, NCH, 128, NMOV]
    x_dev = np.ascontiguousarray(
        x.reshape(NCORES, NCH, NMOV, KT, 128).transpose(0, 3, 1, 4, 2)
        .astype(ml_dtypes.bfloat16)
    )
    # W: [D_out, D_in] -> [og, p(k_local), kt*128 + o], bf16
    w_dev = np.ascontiguousarray(
        W.reshape(G, 128, KT, 128).transpose(0, 3, 2, 1).reshape(G, 128, D)
        .astype(ml_dtypes.bfloat16)
    )
    wg_dev = np.ascontiguousarray(
        Wg.transpose(2, 0, 1).reshape(128, G * IG).astype(ml_dtypes.bfloat16)
    )
    b_dev = np.ascontiguousarray(b.reshape(G, 128).T)
    bg_dev = np.ascontiguousarray(bg.T)

    in_maps = [
        {"x": x_dev[c], "w": w_dev, "wg": wg_dev, "b": b_dev, "bg": bg_dev}
        for c in range(NCORES)
    ]
    nc = _get_nc()
    res = bass_utils.run_bass_kernel_spmd(
        nc, in_maps, core_ids=list(range(NCORES)), trace=trace, tmpdir=tmpdir
    )
    _CACHE["last_result"] = res

    out_t = np.concatenate(
        [res.results[c]["o"].reshape(D, TPC) for c in range(NCORES)], axis=1
    )
    return np.ascontiguousarray(out_t.T).reshape(B, S, D)


def kernel(x, W, b, Wg, bg):
    return _run(x, W, b, Wg, bg, trace=False)
